# revision 1
# baseline (speedup 1.0000x reference)
"""CrossFusionBlock Trainium2 kernel.

Dual-stream cross-attention block (B=8, C=512, HW=1024, 8 heads, FFN 2048).
Sharding: data-parallel over batch across 8 NeuronCores (1 batch element per
core), weights replicated. All weight transposes / bf16 casts are done on the
host so the device kernel contains no transposes at all.

Per-core dataflow (channels-first activations, [C->4x128 partitions, HW]):
  Q_cf = Wq @ X_q        (lhsT = host-supplied Wq^T, rhs = X bf16)
  K_cf = Wk @ X_kv
  V_tok = X_kv^T @ Wv^T  (token-major, lhsT = X bf16) + ones column
  S^T[tk,tq] = K_cf_head^T-slice x Q_cf_head   (K=64, auto row-tiled pairs)
  P^T = exp(S^T/8)  (no max subtraction: logits are O(1) by construction)
  AV: psum[0:64] = O_cf_head, psum[64] = Z (softmax denominator, ones column)
  O /= Z  (GPSIMD partition-broadcast of 1/Z)
  enh = Wo @ O + bo  (per-head K=64 contraction, head-major Wo^T from host)
  LN over channels via PE ones-matmul stats + GPSIMD row broadcast
  FFN: W2 @ gelu(W1 @ s + b1) + b2, residual, LN2.
"""

import sys

import numpy as np

for _p in ("/opt/trn_rl_repo", "/opt/pypackages"):
    if _p not in sys.path:
        sys.path.insert(0, _p)

import ml_dtypes  # noqa: E402

import concourse.bass as bass  # noqa: E402
from concourse import bacc  # noqa: E402
import concourse.mybir as mybir  # noqa: E402
import concourse.tile as tile  # noqa: E402

P = 128
C = 512
HW = 1024
HEADS = 8
DH = 64
HID = 2048
CT = C // P        # 4 channel tiles
HT = HID // P      # 16 hidden tiles
TT = HW // P       # 8 token tiles
NCH = HW // 512    # 2 free-dim chunks of 512
EPS = 1e-6
BF16 = mybir.dt.bfloat16
FP8 = mybir.dt.float8e4
F32 = mybir.dt.float32
AF = mybir.ActivationFunctionType
ALU = mybir.AluOpType

N_CORES = 8
B, H_IMG, W_IMG = 8, 32, 32


# --------------------------------------------------------------------------
# device program
# --------------------------------------------------------------------------

def _emit_proj_one(tc, pools, x_bf, w, out_cf):
    nc = tc.nc
    psum_mm = pools["psum_mm"]
    for ct in range(CT):
        for ch in range(NCH):
            pq = psum_mm.tile([P, 512], F32, tag="mm", name="mm")
            for k in range(CT):
                nc.tensor.matmul(
                    pq,
                    lhsT=w[:, k, ct * P:(ct + 1) * P],
                    rhs=x_bf[:, k, ch * 512:(ch + 1) * 512],
                    start=(k == 0), stop=(k == CT - 1),
                )
            nc.vector.tensor_copy(out=out_cf[:, ct, ch * 512:(ch + 1) * 512], in_=pq)


def _emit_proj_qk(tc, pools, xs_bf, xf_bf, wq, wk, q_cf, k_cf):
    _emit_proj_one(tc, pools, xs_bf, wq, q_cf)
    _emit_proj_one(tc, pools, xf_bf, wk, k_cf)


def _emit_proj_v(tc, pools, xf_bf, wv, v_hf):
    nc = tc.nc
    psum_mm = pools["psum_mm"]
    for tt in range(TT):
        pv = psum_mm.tile([P, 512], F32, tag="mm", name="mm")
        for k in range(CT):
            nc.tensor.matmul(
                pv,
                lhsT=xf_bf[:, k, tt * P:(tt + 1) * P],
                rhs=wv[:, k, :],
                start=(k == 0), stop=(k == CT - 1),
            )
        nc.vector.tensor_copy(
            out=v_hf[:, tt, :, 0:DH],
            in_=pv.rearrange("p (h d) -> p h d", d=DH),
        )
        nc.vector.memset(v_hf[:, tt, :, DH:DH + 1], 1.0)


VW = 72  # V row width: DH + ones col + zero pad (16B-aligned for DoubleRow)


def _emit_st_exp(tc, pools, hp, q_cf, k_cf, filler=None):
    """S^T (row-tiled K=64 pair) -> exp(P^T) in fp8. Returns per-parity PT."""
    nc = tc.nc
    pt = {}
    for par in (0, 1):
        pt[par] = pools["pt"].tile([P, TT, HW], FP8, tag="pt", name="pt", bufs=3)
    ps = {}
    for tt in range(TT):
        if filler is not None:
            filler()
        for par in (0, 1):
            base = par * DH
            p_s = pools["psum_s"].tile([P, HW], F32, tag="s", name="s")
            for ch in range(NCH):
                nc.tensor.matmul(
                    p_s[:, ch * 512:(ch + 1) * 512],
                    lhsT=k_cf[base:base + DH, hp, tt * P:(tt + 1) * P],
                    rhs=q_cf[base:base + DH, hp, ch * 512:(ch + 1) * 512],
                    start=True, stop=True,
                )
            ps[par] = p_s
        for par in (0, 1):
            nc.scalar.activation(out=pt[par][:, tt, :], in_=ps[par],
                                 func=AF.Exp, scale=0.125)
    return pt


def _emit_av(tc, pools, hp, pt, v_hf, o_pair, filler=None):
    """AV+Z (ones column) in fp8 DoubleRow -> normalize into o_pair[:, hp]."""
    nc = tc.nc
    for par in (0, 1):
        h = 2 * hp + par
        for ch in range(NCH):
            if filler is not None:
                filler()
            sl = slice(ch * 512, (ch + 1) * 512)
            pav = pools["psum_av"].tile([VW, 512], F32, tag="av", name="av")
            for tt2 in range(TT // 2):
                nc.tensor.matmul(
                    pav,
                    lhsT=v_hf[:, 2 * tt2:2 * tt2 + 2, h, :],
                    rhs=pt[par][:, 2 * tt2:2 * tt2 + 2, sl],
                    start=(tt2 == 0), stop=(tt2 == TT // 2 - 1),
                    perf_mode=mybir.MatmulPerfMode.DoubleRow,
                )
            rz = pools["rz"].tile([P, 512], F32, tag="rz", name="rz", bufs=2)
            nc.vector.reciprocal(out=rz[DH:DH + 1, :], in_=pav[DH:DH + 1, :])
            nc.sync.dma_start(
                out=rz[0:DH, :],
                in_=rz[DH:DH + 1, None, :].to_broadcast((1, DH, 512)),
            )
            if par == 0:
                nc.vector.tensor_tensor(
                    o_pair[0:DH, hp, sl], pav[0:DH, :], rz[0:DH, :], ALU.mult
                )
            else:
                o_tmp = pools["rz"].tile([DH, 512], FP8, tag="o_tmp",
                                         name="o_tmp", bufs=2)
                nc.vector.tensor_tensor(o_tmp, pav[0:DH, :], rz[0:DH, :], ALU.mult)
                nc.sync.dma_start(out=o_pair[DH:P, hp, sl], in_=o_tmp)


def _emit_layernorm(tc, pools, src_bf, w_ap, b_ap, out_writer, inv512, eps_sb,
                    chunks=tuple(range(NCH)), sub_eng=None):
    """LN over the channel (partition x 4-tile) axis of src_bf [P, CT, HW].

    Pipelined per 512-wide chunk: stats matmuls -> row math -> DMA broadcast
    -> per-ct normalize. out_writer(ct, sl, tile_ap, w, b) consumes each
    normalized [P, 512] piece.
    """
    nc = tc.nc
    psum_mm = pools["psum_mm"]
    for ch in chunks:
        sl = slice(ch * 512, (ch + 1) * 512)
        pmu = psum_mm.tile([1, 512], F32, tag="mm", name="mm")
        for k in range(CT):
            nc.tensor.matmul(
                pmu, lhsT=inv512[:, 0:1], rhs=src_bf[:, k, sl],
                start=(k == 0), stop=(k == CT - 1),
            )
        pms = psum_mm.tile([1, 512], F32, tag="mm", name="mm")
        for k in range(CT):
            r2 = pools["sq"].tile([P, 512], BF16, tag="sq", name="sq")
            nc.gpsimd.tensor_tensor(r2, src_bf[:, k, sl], src_bf[:, k, sl], ALU.mult)
            nc.tensor.matmul(
                pms, lhsT=inv512[:, 0:1], rhs=r2,
                start=(k == 0), stop=(k == CT - 1),
            )
        mu_row = pools["rows"].tile([1, 512], F32, tag="mu_row", name="mu_row", bufs=2)
        rs_row = pools["rows"].tile([1, 512], F32, tag="rs_row", name="rs_row", bufs=2)
        nc.vector.tensor_copy(out=mu_row, in_=pmu)
        musq = pools["rows"].tile([1, 512], F32, tag="musq", name="musq", bufs=1)
        nc.vector.tensor_tensor(musq, mu_row, mu_row, ALU.mult)
        # var = E[x^2] - mu^2 ; rs = 1/sqrt(var + eps)
        nc.vector.tensor_tensor(rs_row, pms, musq, ALU.subtract)
        nc.scalar.activation(rs_row, rs_row, AF.Sqrt, bias=eps_sb[:, 0:1])
        nc.vector.reciprocal(out=rs_row, in_=rs_row)
        mu_b = pools["bcast"].tile([P, 512], F32, tag="mu_b", name="mu_b", bufs=1)
        rs_b = pools["bcast"].tile([P, 512], F32, tag="rs_b", name="rs_b", bufs=1)
        nc.sync.dma_start(out=mu_b, in_=mu_row[0:1, None, :].to_broadcast((1, P, 512)))
        nc.sync.dma_start(out=rs_b, in_=rs_row[0:1, None, :].to_broadcast((1, P, 512)))
        for ct in range(CT):
            tmp = pools["tmp"].tile([P, 512], F32, tag="tmp", name="tmp", bufs=2)
            se = sub_eng if sub_eng is not None else nc.vector
            se.tensor_tensor(tmp, src_bf[:, ct, sl], mu_b, ALU.subtract)
            nc.vector.tensor_tensor(tmp, tmp, rs_b, ALU.mult)
            out_writer(ct, sl, tmp, w_ap(ct), b_ap(ct))


def _emit_wo_residual(tc, pools, pfx, io, cts):
    """Wo projection + bias + residual for the given ct tiles -> r_bf."""
    nc = tc.nc
    o_hf = io["o"]
    x32, wo, params = io["x32"], io["wo"], io["params"]
    psum_mm = pools["psum_mm"]
    if "r" not in io:
        io["r"] = pools["r_pool"].tile([P, CT, HW], BF16, tag=f"r_{pfx}",
                                       name=f"r_{pfx}")
    r_bf = io["r"]
    for ct in cts:
        xr = pools["xr"].tile([P, HW], F32, tag="xr", name="xr")
        nc.sync.dma_start(out=xr, in_=x32[ct * P:(ct + 1) * P, :])
        for ch in range(NCH):
            sl = slice(ch * 512, (ch + 1) * 512)
            pe_ = psum_mm.tile([P, 512], F32, tag="mm", name="mm")
            for i2 in range(HEADS // 4):
                nc.tensor.matmul(
                    pe_,
                    lhsT=wo[:, 2 * i2:2 * i2 + 2, ct * P:(ct + 1) * P],
                    rhs=o_hf[:, 2 * i2:2 * i2 + 2, sl],
                    start=(i2 == 0), stop=(i2 == HEADS // 4 - 1),
                    perf_mode=mybir.MatmulPerfMode.DoubleRow,
                )
            nc.vector.scalar_tensor_tensor(
                out=r_bf[:, ct, sl], in0=pe_, scalar=params["bo"][:, ct:ct + 1],
                in1=xr[:, sl], op0=ALU.add, op1=ALU.add,
            )


def _emit_ln1(tc, pools, pfx, io, chunks=tuple(range(NCH)), sub_eng=None):
    nc = tc.nc
    params = io["params"]
    if "s" not in io:
        io["s"] = pools["s_pool"].tile([P, CT, HW], BF16, tag=f"s_{pfx}",
                                       name=f"s_{pfx}")
    s_bf = io["s"]

    def _ln1_write(ct, sl, tmp, w_scalar, b_scalar):
        nc.vector.tensor_scalar(
            out=s_bf[:, ct, sl], in0=tmp, scalar1=w_scalar, scalar2=b_scalar,
            op0=ALU.mult, op1=ALU.add,
        )

    _emit_layernorm(
        tc, pools, io["r"],
        lambda ct: params["n1w"][:, ct:ct + 1], lambda ct: params["n1b"][:, ct:ct + 1],
        _ln1_write, io["inv512"], io["eps"], chunks, sub_eng=sub_eng,
    )


def _ffn_chunk_pieces(tc, pools, pfx, io, ch):
    """Thunks emitting the FFN chunk piecewise (16 FFN1-ht + 4 FFN2-ct)."""
    nc = tc.nc
    params = io["params"]
    w1, w2 = io["w1"], io["w2"]
    psum_mm = pools["psum_mm"]
    sl = slice(ch * 512, (ch + 1) * 512)
    state = {}

    def ffn1_piece(ht):
        def f():
            if "h" not in state:
                state["h"] = pools["hbuf"].tile([P, HT, 512], BF16, tag="hbuf",
                                                name="hbuf")
            h_ch = state["h"]
            ph = psum_mm.tile([P, 512], F32, tag="mm", name="mm")
            for k in range(CT):
                nc.tensor.matmul(
                    ph,
                    lhsT=w1[:, k, ht * P:(ht + 1) * P],
                    rhs=io["s"][:, k, sl],
                    start=(k == 0), stop=(k == CT - 1),
                )
            nc.scalar.activation(
                out=h_ch[:, ht, :], in_=ph, func=AF.Gelu,
                bias=params["b1"][:, ht:ht + 1],
            )
        return f

    def ffn2_piece(ct):
        def f():
            if "r2" not in io:
                io["r2"] = pools["r_pool"].tile([P, CT, HW], BF16, tag=f"r_{pfx}",
                                                name=f"r2_{pfx}")
            r2_bf = io["r2"]
            h_ch = state["h"]
            pf = psum_mm.tile([P, 512], F32, tag="mm", name="mm")
            for k in range(HT):
                nc.tensor.matmul(
                    pf,
                    lhsT=w2[:, k, ct * P:(ct + 1) * P],
                    rhs=h_ch[:, k, :],
                    start=(k == 0), stop=(k == HT - 1),
                )
            nc.vector.scalar_tensor_tensor(
                out=r2_bf[:, ct, sl], in0=pf, scalar=params["b2"][:, ct:ct + 1],
                in1=io["s"][:, ct, sl], op0=ALU.add, op1=ALU.add,
            )
        return f

    return [ffn1_piece(ht) for ht in range(HT)] + [ffn2_piece(ct) for ct in range(CT)]


def _emit_ffn_chunk(tc, pools, pfx, io, ch):
    """FFN + residual for one 512-wide chunk -> r2_bf."""
    nc = tc.nc
    params = io["params"]
    w1, w2 = io["w1"], io["w2"]
    s_bf = io["s"]
    psum_mm = pools["psum_mm"]
    if "r2" not in io:
        io["r2"] = pools["r_pool"].tile([P, CT, HW], BF16, tag=f"r_{pfx}",
                                        name=f"r2_{pfx}")
    r2_bf = io["r2"]
    sl = slice(ch * 512, (ch + 1) * 512)
    h_ch = pools["hbuf"].tile([P, HT, 512], BF16, tag="hbuf", name="hbuf")
    for ht in range(HT):
        ph = psum_mm.tile([P, 512], F32, tag="mm", name="mm")
        for k in range(CT):
            nc.tensor.matmul(
                ph,
                lhsT=w1[:, k, ht * P:(ht + 1) * P],
                rhs=s_bf[:, k, sl],
                start=(k == 0), stop=(k == CT - 1),
            )
        nc.scalar.activation(
            out=h_ch[:, ht, :], in_=ph, func=AF.Gelu,
            bias=params["b1"][:, ht:ht + 1],
        )
    for ct in range(CT):
        pf = psum_mm.tile([P, 512], F32, tag="mm", name="mm")
        for k in range(HT):
            nc.tensor.matmul(
                pf,
                lhsT=w2[:, k, ct * P:(ct + 1) * P],
                rhs=h_ch[:, k, :],
                start=(k == 0), stop=(k == HT - 1),
            )
        nc.vector.scalar_tensor_tensor(
            out=r2_bf[:, ct, sl], in0=pf, scalar=params["b2"][:, ct:ct + 1],
            in1=s_bf[:, ct, sl], op0=ALU.add, op1=ALU.add,
        )


def _emit_ln2(tc, pools, pfx, io, chunks=tuple(range(NCH)), sub_eng=None):
    nc = tc.nc
    params, out_dram = io["params"], io["out"]

    def _ln2_write(ct, sl, tmp, w_scalar, b_scalar):
        o32 = pools["ostage"].tile([P, 512], F32, tag="ostage", name="ostage", bufs=2)
        nc.vector.tensor_scalar(
            out=o32, in0=tmp, scalar1=w_scalar, scalar2=b_scalar,
            op0=ALU.mult, op1=ALU.add,
        )
        nc.sync.dma_start(out=out_dram[ct * P:(ct + 1) * P, sl], in_=o32)

    _emit_layernorm(
        tc, pools, io["r2"],
        lambda ct: params["n2w"][:, ct:ct + 1], lambda ct: params["n2b"][:, ct:ct + 1],
        _ln2_write, io["inv512"], io["eps"], chunks, sub_eng=sub_eng,
    )


def build_program():
    nc = bacc.Bacc("TRN2", target_bir_lowering=False, debug=False)

    def din(name, shape, dt):
        return nc.dram_tensor(name, list(shape), dt, kind="ExternalInput").ap()

    x32 = {p: din(f"x_{p}32", (C, HW), F32) for p in "sf"}
    xbf = {p: din(f"x_{p}bf", (C, HW), BF16) for p in "sf"}
    wqt = {p: din(f"{p}_wqt", (C, C), BF16) for p in "sf"}
    wkt = {p: din(f"{p}_wkt", (C, C), BF16) for p in "sf"}
    wvt = {p: din(f"{p}_wvt", (C, C), BF16) for p in "sf"}
    wot = {p: din(f"{p}_wot", (C, C), FP8) for p in "sf"}
    w1t = {p: din(f"{p}_w1t", (C, HID), BF16) for p in "sf"}
    w2t = {p: din(f"{p}_w2t", (HID, C), BF16) for p in "sf"}
    pnames = ("bo", "n1w", "n1b", "n2w", "n2b", "b2")
    prm = {
        p: {n: din(f"{p}_{n}", (P, CT), F32) for n in pnames} for p in "sf"
    }
    for p in "sf":
        prm[p]["b1"] = din(f"{p}_b1", (P, HT), F32)
    outs = {
        p: nc.dram_tensor(f"out_{p}", [C, HW], F32, kind="ExternalOutput").ap()
        for p in "sf"
    }

    with tile.TileContext(nc) as tc:
        from contextlib import ExitStack
        with ExitStack() as ctx:
            pools = {}

            def pool(name, bufs, space="SBUF", stack=None):
                pools[name] = (stack or ctx).enter_context(
                    tc.tile_pool(name=name, bufs=bufs, space=space)
                )
                return pools[name]

            # whole-program pools
            pool("psum_mm", 2, space="PSUM")
            pool("psum_s", 2, space="PSUM")
            pool("psum_av", 2, space="PSUM")
            pool("consts", 1)
            pool("params", 1)
            pool("xr", 1)
            pool("rows", 1)
            pool("bcast", 1)
            pool("tmp", 1)
            pool("sq", 2)
            pool("rz", 1)
            pool("pt", 34)
            pool("r_pool", 1)
            pool("s_pool", 1)
            pool("hbuf", 1)
            pool("ostage", 2)
            pool("wffn", 1)

            inv512 = pools["consts"].tile([P, 1], BF16)
            nc.vector.memset(inv512, 1.0 / C)
            eps_sb = pools["consts"].tile([1, 1], F32)
            nc.vector.memset(eps_sb, EPS)

            # ---- load params (small) ----
            params = {}
            for p in "sf":
                params[p] = {}
                for n, ap_ in prm[p].items():
                    t = pools["params"].tile(list(ap_.shape), F32, tag=f"{p}_{n}")
                    nc.sync.dma_start(out=t, in_=ap_)
                    params[p][n] = t

            # ---- pools with manual lifetimes (LIFO discipline) ----
            owo_stack = ctx.enter_context(ExitStack())
            pool("o_pool", 1, stack=owo_stack)
            pool("wo_pool", 1, stack=owo_stack)
            qkv_stack = ctx.enter_context(ExitStack())
            pool("qkv", 1, stack=qkv_stack)
            xw_stack = ctx.enter_context(ExitStack())
            pool("xbf", 1, stack=xw_stack)
            pool("wproj", 1, stack=xw_stack)

            def load_wproj(p, nm, srcw):
                t = pools["wproj"].tile([P, CT, C], BF16, tag=nm, name=f"{nm}_{p}")
                for ct_ in range(CT):
                    eng = (nc.gpsimd, nc.scalar, nc.sync, nc.gpsimd)[ct_ % 4]
                    eng.dma_start(
                        out=t[:, ct_, :], in_=srcw[ct_ * P:(ct_ + 1) * P, :]
                    )
                return t

            def load_xbf(p):
                t = pools["xbf"].tile([P, CT, HW], BF16, tag=f"xbf_{p}",
                                      name=f"xbf_{p}")
                for ct_ in range(CT):
                    eng = (nc.sync, nc.gpsimd, nc.scalar, nc.sync)[ct_ % 4]
                    eng.dma_start(
                        out=t[:, ct_, :], in_=xbf[p][ct_ * P:(ct_ + 1) * P, :]
                    )
                return t

            # Q(s) needs only x_s + wq_s: emit those DMAs first so the first
            # projection matmuls start ~1.3MB into the input stream, not 3.5MB.
            xbf_sb = {"s": load_xbf("s")}
            wq_s = load_wproj("s", "wq", wqt["s"])
            xbf_sb["f"] = load_xbf("f")

            qkv = {}
            for p in "sf":
                qkv[f"q_{p}"] = pools["qkv"].tile(
                    [P, CT, HW], FP8, tag=f"q_{p}", name=f"q_{p}")
                qkv[f"k_{p}"] = pools["qkv"].tile(
                    [P, CT, HW], FP8, tag=f"k_{p}", name=f"k_{p}")
                qkv[f"v_{p}"] = pools["qkv"].tile(
                    [P, TT, HEADS, VW], FP8, tag=f"v_{p}", name=f"v_{p}")
                nc.vector.memset(qkv[f"v_{p}"][:, :, :, DH + 1:], 0.0)

            wo_sb = {}
            o_sb = {}
            for p in "sf":
                wo_sb[p] = pools["wo_pool"].tile([P, CT, C], FP8, tag=f"wo_{p}",
                                                 name=f"wo_{p}")
                o_sb[p] = pools["o_pool"].tile([P, HEADS // 2, HW], FP8,
                                               tag=f"o_{p}", name=f"o_{p}")

            def load_wo(p):
                nc.sync.dma_start(
                    out=wo_sb[p],
                    in_=wot[p].rearrange("(ct p) o -> p ct o", p=P),
                )

            ios = {}
            for p in "sf":
                ios[p] = {
                    "o": o_sb[p], "x32": x32[p], "wo": wo_sb[p],
                    "params": params[p], "out": outs[p],
                    "inv512": inv512, "eps": eps_sb,
                }

            # software-pipelined attention: S^T+exp of pair N overlaps
            # AV of pair N-1 on PE, so PE never waits on the ACT exp chain.
            # stream 's': q from x_s, kv from x_f ; stream 'f': swapped
            seq = [("s", hp) for hp in range(4)] + [("f", hp) for hp in range(4)]
            pts = {}

            def st(i):
                p, hp = seq[i]
                pts[i] = _emit_st_exp(tc, pools, hp, qkv[f"q_{p}"], qkv[f"k_{p}"])

            def av(i):
                p, hp = seq[i]
                _emit_av(tc, pools, hp, pts.pop(i), qkv[f"v_{p}"], o_sb[p])

            # ---- A(s) ----
            _emit_proj_qk(tc, pools, xbf_sb["s"], xbf_sb["f"],
                          wq_s,
                          load_wproj("s", "wk", wkt["s"]),
                          qkv["q_s"], qkv["k_s"])
            _emit_proj_v(tc, pools, xbf_sb["f"], load_wproj("s", "wv", wvt["s"]),
                         qkv["v_s"])

            # ---- B(s) | A(f) ----
            st(0)
            _emit_proj_qk(tc, pools, xbf_sb["f"], xbf_sb["s"],
                          load_wproj("f", "wq", wqt["f"]),
                          load_wproj("f", "wk", wkt["f"]),
                          qkv["q_f"], qkv["k_f"])
            st(1)
            av(0)
            _emit_proj_v(tc, pools, xbf_sb["s"], load_wproj("f", "wv", wvt["f"]),
                         qkv["v_f"])
            load_wo("s")
            st(2)
            av(1)
            load_wo("f")
            st(3)
            av(2)
            xw_stack.close()

            def load_wffn(p):
                t1 = pools["wffn"].tile([P, CT, HID], BF16, tag="w1", name="w1")
                for ct_ in range(CT):
                    eng = (nc.sync, nc.gpsimd, nc.scalar, nc.sync)[ct_ % 4]
                    eng.dma_start(
                        out=t1[:, ct_, :], in_=w1t[p][ct_ * P:(ct_ + 1) * P, :]
                    )
                t2 = pools["wffn"].tile([P, HT, C], BF16, tag="w2", name="w2")
                for g in range(4):
                    eng = (nc.gpsimd, nc.scalar, nc.sync, nc.gpsimd)[g % 4]
                    eng.dma_start(
                        out=t2[:, 4 * g:4 * (g + 1), :],
                        in_=w2t[p][4 * g * P:4 * (g + 1) * P, :].rearrange(
                            "(ht p) o -> p ht o", p=P),
                    )
                return t1, t2

            ios["s"]["w1"], ios["s"]["w2"] = load_wffn("s")

            # ---- B(f) | C(s) | D(s) ----
            st(4)
            av(3)
            _emit_wo_residual(tc, pools, "s", ios["s"], (0, 1))
            st(5)
            av(4)
            _emit_wo_residual(tc, pools, "s", ios["s"], (2, 3))
            st(6)
            av(5)
            _emit_ln1(tc, pools, "s", ios["s"], chunks=(0,))
            st(7)
            av(6)
            _emit_ln1(tc, pools, "s", ios["s"], chunks=(1,))
            _emit_ffn_chunk(tc, pools, "s", ios["s"], 0)
            av(7)
            _emit_ffn_chunk(tc, pools, "s", ios["s"], 1)
            qkv_stack.close()

            # ---- C(f) | LN2(s); then D(f) ----
            _emit_wo_residual(tc, pools, "f", ios["f"], (0, 1))
            _emit_wo_residual(tc, pools, "f", ios["f"], (2, 3))
            _emit_ln1(tc, pools, "f", ios["f"], chunks=(0,))
            _emit_ln2(tc, pools, "s", ios["s"], chunks=(0,), sub_eng=nc.gpsimd)
            _emit_ln1(tc, pools, "f", ios["f"], chunks=(1,))
            ios["f"]["w1"], ios["f"]["w2"] = load_wffn("f")
            _emit_ln2(tc, pools, "s", ios["s"], chunks=(1,), sub_eng=nc.gpsimd)
            _emit_ffn_chunk(tc, pools, "f", ios["f"], 0)
            _emit_ln2(tc, pools, "f", ios["f"], chunks=(0,), sub_eng=nc.gpsimd)
            _emit_ffn_chunk(tc, pools, "f", ios["f"], 1)
            _emit_ln2(tc, pools, "f", ios["f"], chunks=(1,), sub_eng=nc.gpsimd)

    nc.compile()
    return nc


# --------------------------------------------------------------------------
# host side
# --------------------------------------------------------------------------

_BF = ml_dtypes.bfloat16
_F8 = ml_dtypes.float8_e4m3


def _prep_shared_inputs(inputs):
    """Host-side weight prep: transposes, bf16 casts, per-partition layouts."""
    sh = {}
    for p, ap in (("s", "s_"), ("f", "f_")):
        wq, wk, wv, wo = (inputs[ap + n] for n in ("Wq", "Wk", "Wv", "Wo"))
        sh[f"{p}_wqt"] = np.ascontiguousarray(wq.T).astype(_BF)
        sh[f"{p}_wkt"] = np.ascontiguousarray(wk.T).astype(_BF)
        sh[f"{p}_wvt"] = np.ascontiguousarray(wv.T).astype(_BF)
        sh[f"{p}_wot"] = np.ascontiguousarray(wo.T).astype(_F8)
        w1 = inputs[f"{p}ffn_W1"]
        w2 = inputs[f"{p}ffn_W2"]
        sh[f"{p}_w1t"] = np.ascontiguousarray(w1.T).astype(_BF)
        sh[f"{p}_w2t"] = np.ascontiguousarray(w2.T).astype(_BF)
        sh[f"{p}_bo"] = np.ascontiguousarray(
            inputs[ap + "bo"].reshape(CT, P).T
        ).astype(np.float32)
        n1w, n1b = (f"{p}n1_w", f"{p}n1_b")
        n2w, n2b = (f"{p}n2_w", f"{p}n2_b")
        sh[f"{p}_n1w"] = np.ascontiguousarray(inputs[n1w].reshape(CT, P).T).astype(np.float32)
        sh[f"{p}_n1b"] = np.ascontiguousarray(inputs[n1b].reshape(CT, P).T).astype(np.float32)
        sh[f"{p}_n2w"] = np.ascontiguousarray(inputs[n2w].reshape(CT, P).T).astype(np.float32)
        sh[f"{p}_n2b"] = np.ascontiguousarray(inputs[n2b].reshape(CT, P).T).astype(np.float32)
        sh[f"{p}_b1"] = np.ascontiguousarray(
            inputs[f"{p}ffn_b1"].reshape(HT, P).T
        ).astype(np.float32)
        sh[f"{p}_b2"] = np.ascontiguousarray(
            inputs[f"{p}ffn_b2"].reshape(CT, P).T
        ).astype(np.float32)
    return sh


def _rename_ln(inputs):
    """Map reference param names (sn1_w...) onto the scheme used above."""
    out = dict(inputs)
    for p in "sf":
        for i in "12":
            for wb in "wb":
                out[f"{p}n{i}_{wb}"] = inputs[f"{p}n{i}_{wb}"]
    return out


def make_in_maps(inputs):
    inputs = _rename_ln(inputs)
    shared = _prep_shared_inputs(inputs)
    xs = np.ascontiguousarray(inputs["spatial_feat"].reshape(B, C, HW))
    xf = np.ascontiguousarray(inputs["freq_feat"].reshape(B, C, HW))
    in_maps = []
    for b in range(N_CORES):
        m = dict(shared)
        m["x_s32"] = np.ascontiguousarray(xs[b]).astype(np.float32)
        m["x_f32"] = np.ascontiguousarray(xf[b]).astype(np.float32)
        m["x_sbf"] = xs[b].astype(_BF)
        m["x_fbf"] = xf[b].astype(_BF)
        in_maps.append(m)
    return in_maps


_CACHED = {}


def _get_program():
    if "nc" not in _CACHED:
        _CACHED["nc"] = build_program()
    return _CACHED["nc"]


def run_on_hw(inputs, trace=False, trace_kwargs=None):
    from concourse.bass_utils import run_bass_kernel_spmd

    nc = _get_program()
    in_maps = make_in_maps(inputs)
    res = run_bass_kernel_spmd(
        nc, in_maps, list(range(N_CORES)), trace=trace,
        **(dict(trace_kwargs=trace_kwargs) if trace_kwargs else {}),
    )
    s = np.stack([res.results[b]["out_s"] for b in range(B)])
    f = np.stack([res.results[b]["out_f"] for b in range(B)])
    s = s.reshape(B, C, H_IMG, W_IMG).astype(np.float32)
    f = f.reshape(B, C, H_IMG, W_IMG).astype(np.float32)
    return (s, f), res


def kernel(**inputs):
    out, _ = run_on_hw(inputs, trace=False)
    return out



# revision 15
# speedup vs baseline: 1.3711x; 1.3711x over previous
"""CrossFusionBlock Trainium2 kernel (v2).

Dual-stream cross-attention block (B=8, C=512, HW=1024, 8 heads, FFN 2048).
Sharding: data-parallel over batch across 8 NeuronCores (1 batch element per
core), weights replicated. All transposes / dtype casts / layouts on host.

Per-core dataflow:
  Q/K/V proj: fp8 DoubleRow (x fp8, weights fp8 x16), chains of 2 ct-pairs.
  S^T:        per head-pair: lhsT = k_cf[base:base+64, hp, tok128] (fp8,
              K=64, walrus row-tiles the par0/par1 pairs), logits x256 from
              the weight scaling folded into the exp scale.
  P^T = exp(S^T * 0.125/256) fp8 via ACT (the bottleneck engine: 128 x
              [128,1024] exps = 133us irreducible on ACT).
  AV:         fp8 DoubleRow over tt-pairs, ones column = 16 (folds away the
              Wv x16), psum rows 0:64 = O, row 64 = 16*Z; O /= Z via DVE
              reciprocal + DMA partition-broadcast.
  Wo:         head-major fp8 DoubleRow (unscaled), + bias + bf16-x residual.
  LN:         stats via ones[128,128] matmul -> broadcast psum [128,512]
              (mu: bf16 chain 4; E[x^2]: fp8 r^2 + DoubleRow chain 2 with
              1/64 weights, 1/8 compensation in the var STT); bf16 row math
              (2x DVE); sqrt on ACT; 3-op normalize.
  FFN:        fp8 DoubleRow both layers; W1 x16 folded into gelu scale;
              h fp8; LN2(s) deferred to the tail so ACT table loads cluster
              (exp/sqrt/gelu switches cost 1283ns each).
"""

import sys

import numpy as np

for _p in ("/opt/trn_rl_repo", "/opt/pypackages"):
    if _p not in sys.path:
        sys.path.insert(0, _p)

import ml_dtypes  # noqa: E402

import concourse.bass as bass  # noqa: E402
from concourse import bacc  # noqa: E402
import concourse.mybir as mybir  # noqa: E402
import concourse.tile as tile  # noqa: E402

P = 128
C = 512
HW = 1024
HEADS = 8
DH = 64
HID = 2048
CT = C // P        # 4 channel tiles
HT = HID // P      # 16 hidden tiles
TT = HW // P       # 8 token tiles
NCH = HW // 512    # 2 free-dim chunks of 512
EPS = 1e-6
BF16 = mybir.dt.bfloat16
FP8 = mybir.dt.float8e4
F32 = mybir.dt.float32
AF = mybir.ActivationFunctionType
ALU = mybir.AluOpType
DR = mybir.MatmulPerfMode.DoubleRow

N_CORES = 8
B, H_IMG, W_IMG = 8, 32, 32

WSCALE = 16.0               # host scale on Wq, Wk, Wv, W1
EXP_SCALE = 0.125 / (WSCALE * WSCALE)
VW = 72                     # v row pitch (keeps tt-pair stride 16B-aligned)

# packed param column offsets (per stream base = 40 * stream_index)
PCOL = {"bo": 0, "n1w": 4, "n1b": 8, "n2w": 12, "n2b": 16, "b2": 20, "b1": 24}
NPCOL = 40


# --------------------------------------------------------------------------
# device program
# --------------------------------------------------------------------------


def build_program():
    nc = bacc.Bacc("TRN2", target_bir_lowering=False, debug=False)

    def din(name, shape, dt):
        return nc.dram_tensor(name, list(shape), dt, kind="ExternalInput").ap()

    x8 = {p: din(f"x_{p}8", (C, HW), FP8) for p in "sf"}
    xbf = {p: din(f"x_{p}bf", (C, HW), BF16) for p in "sf"}
    wq8 = {p: din(f"{p}_wq8", (P, 2 * 2 * C), FP8) for p in "sf"}
    wk8 = {p: din(f"{p}_wk8", (P, 2 * 2 * C), FP8) for p in "sf"}
    wv8 = {p: din(f"{p}_wv8", (P, 2 * 2 * C), FP8) for p in "sf"}
    wot = {p: din(f"{p}_wot", (C, C), FP8) for p in "sf"}
    w18 = {p: din(f"{p}_w18", (P, 2 * 2 * HID), FP8) for p in "sf"}
    w28 = {p: din(f"{p}_w28", (P, 8 * 2 * C), FP8) for p in "sf"}
    prm = din("prm", (P, 2 * NPCOL), F32)
    outs = {
        p: nc.dram_tensor(f"out_{p}", [C, HW], F32, kind="ExternalOutput").ap()
        for p in "sf"
    }

    with tile.TileContext(nc) as tc:
        from contextlib import ExitStack
        with ExitStack() as ctx:
            pools = {}

            def pool(name, bufs, space="SBUF", stack=None):
                pools[name] = (stack or ctx).enter_context(
                    tc.tile_pool(name=name, bufs=bufs, space=space)
                )
                return pools[name]

            pool("psum_mm", 2, space="PSUM")   # [128,512] f32: proj/stats/ffn
            pool("psum_s", 2, space="PSUM")    # [128,1024] f32: S^T
            pool("psum_av", 2, space="PSUM")   # [72,512] f32: AV
            pool("consts", 1)
            pool("params", 1)
            pool("rows", 2)       # mu/musq/var/rs [128,512] bf16
            pool("rz", 2)
            pool("pt", 4)         # 2 tiles/unit x (write unit + read unit)
            pool("ostage", 2)
            pool("tmp", 2)
            pool("o_pool", 1)
            pool("wo_pool", 1)

            # consts
            eps_col = pools["consts"].tile([P, 1], F32)
            nc.vector.memset(eps_col, EPS)
            ones_bf = pools["consts"].tile([P, P], BF16)
            nc.vector.memset(ones_bf, 1.0 / C)
            # fp8 stats weight: 1/64 (exact normal); compensated by 1/8 below
            ones_f8 = pools["consts"].tile([P, 2, P], FP8)
            nc.vector.memset(ones_f8, 1.0 / 64)
            inv8_col = pools["consts"].tile([P, 1], F32)
            nc.vector.memset(inv8_col, 1.0 / 8)

            # packed params, one DMA
            params = pools["params"].tile([P, 2 * NPCOL], F32)
            nc.gpsimd.dma_start(out=params, in_=prm)

            def pp(p, name):
                base = (0 if p == "s" else NPCOL) + PCOL[name]
                n = 16 if name == "b1" else 4
                return params[:, base:base + n]

            # everything fits SBUF concurrently (~199KB/partition), so no
            # manual pool lifetime management is needed
            for p in "sf":
                pool(f"qkv_{p}", 1)
                pool(f"cd_{p}", 1)
            pool("xbf", 1)
            pool("x8", 1)
            pool("wproj", 1)
            pool("wffn", 1)
            pool("hbuf", 2)

            # ---- input loads (chunked; fp8 x first so proj starts early) ----
            x8_sb = {}

            def load_x8(p):
                t = pools["x8"].tile([P, CT, HW], FP8, tag=f"x8_{p}",
                                     name=f"x8_{p}")
                x8_sb[p] = t
                engs = (nc.gpsimd, nc.sync)
                i = 0
                for ch_ in range(NCH):
                    sl = slice(ch_ * 512, (ch_ + 1) * 512)
                    for ct_ in range(CT):
                        engs[i % 2].dma_start(
                            out=t[:, ct_, sl],
                            in_=x8[p][ct_ * P:(ct_ + 1) * P, sl],
                        )
                        i += 1
                return t

            def load_w(nm, p, src):
                t = pools["wproj"].tile([P, 2, 2, C], FP8, tag=nm,
                                        name=f"{nm}_{p}")
                nc.gpsimd.dma_start(
                    out=t.rearrange("p a b o -> p (a b o)"), in_=src)
                return t

            xbf_sb = {}

            def load_xbf(p):
                # ACT DGE queue: keeps gpsimd/sync queues free for weights
                # and latency-critical broadcasts
                t = pools["xbf"].tile([P, CT, HW], BF16, tag=f"xbf_{p}",
                                      name=f"xbf_{p}")
                for ct_ in range(CT):
                    nc.scalar.dma_start(
                        out=t[:, ct_, :], in_=xbf[p][ct_ * P:(ct_ + 1) * P, :]
                    )
                xbf_sb[p] = t

            # ---- qkv tiles ----
            qkv = {}
            for p in "sf":
                qp = pools[f"qkv_{p}"]
                qkv[f"q_{p}"] = qp.tile([P, CT, HW], FP8,
                                        tag=f"q_{p}", name=f"q_{p}")
                qkv[f"k_{p}"] = qp.tile([P, CT, HW], FP8,
                                        tag=f"k_{p}", name=f"k_{p}")
                qkv[f"v_{p}"] = qp.tile([P, TT, HEADS, VW], FP8,
                                        tag=f"v_{p}", name=f"v_{p}")
                nc.vector.memset(qkv[f"v_{p}"][:, :, :, DH:DH + 1], WSCALE)

            wo_sb = {}
            o_sb = {}
            for p in "sf":
                wo_sb[p] = pools["wo_pool"].tile([P, CT, C], FP8, tag=f"wo_{p}",
                                                 name=f"wo_{p}")
                o_sb[p] = pools["o_pool"].tile([P, HEADS // 2, HW], FP8,
                                               tag=f"o_{p}", name=f"o_{p}")

            # ------------------------------------------------------------------
            # emission helpers
            # ------------------------------------------------------------------

            def emit_proj_qk(xp, w, out_cf):
                """Q or K projection: 8 chains of 2 DR matmuls + copies."""
                for ct in range(CT):
                    for ch in range(NCH):
                        sl = slice(ch * 512, (ch + 1) * 512)
                        pq = pools["psum_mm"].tile([P, 512], F32, tag="mm",
                                                   name="mm")
                        for i in range(2):
                            nc.tensor.matmul(
                                pq,
                                lhsT=w[:, i, :, ct * P:(ct + 1) * P],
                                rhs=xp[:, 2 * i:2 * i + 2, sl],
                                start=(i == 0), stop=(i == 1),
                                perf_mode=DR,
                            )
                        nc.vector.tensor_copy(out=out_cf[:, ct, sl], in_=pq)

            def emit_proj_v(xp, wv, v_hf):
                for tt in range(TT):
                    pv = pools["psum_mm"].tile([P, 512], F32, tag="mm", name="mm")
                    for i in range(2):
                        nc.tensor.matmul(
                            pv,
                            lhsT=xp[:, 2 * i:2 * i + 2, tt * P:(tt + 1) * P],
                            rhs=wv[:, i, :, :],
                            start=(i == 0), stop=(i == 1),
                            perf_mode=DR,
                        )
                    nc.vector.tensor_copy(
                        out=v_hf[:, tt, :, 0:DH],
                        in_=pv.rearrange("p (h d) -> p h d", d=DH),
                    )

            pts = {}

            def st(p, hp, fillers=None):
                """S^T + exp for head pair hp of stream p."""
                q, k = qkv[f"q_{p}"], qkv[f"k_{p}"]
                pt = {}
                for par in (0, 1):
                    pt[par] = pools["pt"].tile([P, TT, HW], FP8, tag="pt",
                                               name=f"pt_{p}{hp}{par}")
                pts[(p, hp)] = pt
                for tt in range(TT):
                    if fillers and tt > 0:   # keep tt0's S^T at the head
                        fillers.pop(0)()
                    ps = {}
                    for par in (0, 1):
                        base = par * DH
                        p_s = pools["psum_s"].tile([P, HW], F32, tag="s",
                                                   name="s")
                        for ch in range(NCH):
                            nc.tensor.matmul(
                                p_s[:, ch * 512:(ch + 1) * 512],
                                lhsT=k[base:base + DH, hp, tt * P:(tt + 1) * P],
                                rhs=q[base:base + DH, hp,
                                      ch * 512:(ch + 1) * 512],
                                start=True, stop=True,
                            )
                        ps[par] = p_s
                    for par in (0, 1):
                        nc.scalar.activation(out=pt[par][:, tt, :], in_=ps[par],
                                             func=AF.Exp, scale=EXP_SCALE)
                while fillers:
                    fillers.pop(0)()

            def av(p, hp):
                """AV + normalize for head pair hp -> o_sb[p]."""
                pt = pts.pop((p, hp))
                v_hf = qkv[f"v_{p}"]
                for par in (0, 1):
                    h = 2 * hp + par
                    for ch in range(NCH):
                        sl = slice(ch * 512, (ch + 1) * 512)
                        pav = pools["psum_av"].tile([VW, 512], F32, tag="av",
                                                    name="av")
                        for t2 in range(TT // 2):
                            nc.tensor.matmul(
                                pav[0:DH + 1, :],
                                lhsT=v_hf[:, 2 * t2:2 * t2 + 2, h, 0:DH + 1],
                                rhs=pt[par][:, 2 * t2:2 * t2 + 2, sl],
                                start=(t2 == 0), stop=(t2 == TT // 2 - 1),
                                perf_mode=DR,
                            )
                        rz = pools["rz"].tile([P, 512], F32, tag="rz", name="rz")
                        nc.vector.reciprocal(out=rz[DH:DH + 1, :],
                                             in_=pav[DH:DH + 1, :])
                        nc.sync.dma_start(
                            out=rz[0:DH, :],
                            in_=rz[DH:DH + 1, None, :].to_broadcast((1, DH, 512)),
                        )
                        if par == 0:
                            nc.vector.tensor_tensor(
                                o_sb[p][0:DH, hp, sl], pav[0:DH, :],
                                rz[0:DH, :], ALU.mult,
                            )
                        else:
                            ot = pools["rz"].tile([DH, 512], FP8, tag="ot",
                                                  name="ot")
                            nc.vector.tensor_tensor(ot, pav[0:DH, :],
                                                    rz[0:DH, :], ALU.mult)
                            nc.sync.dma_start(out=o_sb[p][DH:P, hp, sl], in_=ot)

            ios = {p: {} for p in "sf"}

            def wo_residual_pieces(p):
                """Wo + bias + residual -> r (bf16) + r^2 (fp8, for LN stats)."""
                io = ios[p]
                io["r"] = pools[f"cd_{p}"].tile([P, CT, HW], BF16, tag="r",
                                                name=f"r_{p}")
                io["sq1"] = pools[f"cd_{p}"].tile([P, CT, HW], FP8, tag="sq",
                                                  name=f"sq1_{p}")
                r, sq1 = io["r"], io["sq1"]
                bo = pp(p, "bo")
                out = []
                for ct in range(CT):
                    for ch in range(NCH):
                        def f(ct=ct, ch=ch):
                            sl = slice(ch * 512, (ch + 1) * 512)
                            pe_ = pools["psum_mm"].tile([P, 512], F32, tag="mm",
                                                        name="mm")
                            for i2 in range(HEADS // 4):
                                nc.tensor.matmul(
                                    pe_,
                                    lhsT=wo_sb[p][:, 2 * i2:2 * i2 + 2,
                                                  ct * P:(ct + 1) * P],
                                    rhs=o_sb[p][:, 2 * i2:2 * i2 + 2, sl],
                                    start=(i2 == 0), stop=(i2 == HEADS // 4 - 1),
                                    perf_mode=DR,
                                )
                            nc.vector.scalar_tensor_tensor(
                                out=r[:, ct, sl], in0=pe_,
                                scalar=bo[:, ct:ct + 1],
                                in1=xbf_sb[p][:, ct, sl],
                                op0=ALU.add, op1=ALU.add,
                            )
                            nc.gpsimd.tensor_tensor(
                                sq1[:, ct, sl], r[:, ct, sl], r[:, ct, sl],
                                ALU.mult,
                            )
                        out.append(f)
                return out

            def ln_stats(src_bf, sq_f8, ch):
                """Broadcast-form LN stats for one 512 chunk -> (pmu, pms)."""
                sl = slice(ch * 512, (ch + 1) * 512)
                pmu = pools["psum_mm"].tile([P, 512], F32, tag="mm", name="mm")
                for k in range(CT):
                    nc.tensor.matmul(
                        pmu, lhsT=ones_bf, rhs=src_bf[:, k, sl],
                        start=(k == 0), stop=(k == CT - 1),
                    )
                pms = pools["psum_mm"].tile([P, 512], F32, tag="mm", name="mm")
                for i in range(2):
                    nc.tensor.matmul(
                        pms, lhsT=ones_f8, rhs=sq_f8[:, 2 * i:2 * i + 2, sl],
                        start=(i == 0), stop=(i == 1),
                        perf_mode=DR,
                    )
                return pmu, pms

            def ln_rows(pmu, pms):
                """mu/var row math -> (mu_bf, rs_bf). Frees the psum tiles."""
                mu_bf = pools["rows"].tile([P, 512], BF16, tag="mu", name="mu")
                nc.vector.tensor_copy(out=mu_bf, in_=pmu)
                musq = pools["rows"].tile([P, 512], BF16, tag="musq",
                                          name="musq", bufs=1)
                nc.vector.tensor_tensor(musq, mu_bf, mu_bf, ALU.mult)
                var_bf = pools["rows"].tile([P, 512], BF16, tag="var",
                                            name="var", bufs=1)
                # pms holds 8*E[x^2] (fp8 stats weight is 1/64, true is 1/512)
                nc.vector.scalar_tensor_tensor(
                    out=var_bf, in0=pms, scalar=inv8_col[:, 0:1], in1=musq,
                    op0=ALU.mult, op1=ALU.subtract,
                )
                rs_bf = pools["rows"].tile([P, 512], BF16, tag="rs", name="rs")
                nc.scalar.activation(rs_bf, var_bf, AF.Sqrt, bias=eps_col)
                with nc.allow_low_precision(reason="bf16 rsqrt: 0.4% rel "
                                            "fits the 2e-2 gate"):
                    nc.vector.reciprocal(out=rs_bf, in_=rs_bf)
                return mu_bf, rs_bf

            def ln_norm(src_bf, mu_bf, rs_bf, ch, writer):
                sl = slice(ch * 512, (ch + 1) * 512)
                for ct in range(CT):
                    t1 = pools["tmp"].tile([P, 512], BF16, tag="t1", name="t1")
                    nc.vector.tensor_tensor(t1, src_bf[:, ct, sl], mu_bf,
                                            ALU.subtract)
                    t2 = pools["tmp"].tile([P, 512], BF16, tag="t2", name="t2")
                    nc.vector.tensor_tensor(t2, t1, rs_bf, ALU.mult)
                    writer(ct, sl, t2)

            def ln1_finish(p, ch, pmu, pms):
                io = ios[p]
                if "s" not in io:
                    io["s"] = pools[f"cd_{p}"].tile([P, CT, HW], BF16,
                                                    tag="s", name=f"s_{p}")
                    io["s8"] = pools[f"cd_{p}"].tile([P, CT, HW], FP8,
                                                     tag="s8", name=f"s8_{p}")
                s_bf, s8 = io["s"], io["s8"]
                w_ap, b_ap = pp(p, "n1w"), pp(p, "n1b")
                mu_bf, rs_bf = ln_rows(pmu, pms)

                def writer(ct, sl, t2):
                    nc.vector.tensor_scalar(
                        out=s_bf[:, ct, sl], in0=t2,
                        scalar1=w_ap[:, ct:ct + 1], scalar2=b_ap[:, ct:ct + 1],
                        op0=ALU.mult, op1=ALU.add,
                    )
                    nc.gpsimd.tensor_copy(out=s8[:, ct, sl], in_=s_bf[:, ct, sl])

                ln_norm(io["r"], mu_bf, rs_bf, ch, writer)

            def ffn1_pieces(p, ch):
                """16 thunks: FFN1 DR chain + gelu (W1 x16 folded in scale)."""
                io = ios[p]
                if "h" not in io:
                    io["h"] = pools["hbuf"].tile([P, HT, 512], FP8, tag="hbuf",
                                                 name=f"h_{p}{ch}")
                h = io["h"]
                w1 = io["w1"]
                b1 = pp(p, "b1")
                sl = slice(ch * 512, (ch + 1) * 512)
                out = []
                for ht in range(HT):
                    def f(ht=ht):
                        ph = pools["psum_mm"].tile([P, 512], F32, tag="mm",
                                                   name="mm")
                        for i in range(2):
                            nc.tensor.matmul(
                                ph,
                                lhsT=w1[:, i, :, ht * P:(ht + 1) * P],
                                rhs=io["s8"][:, 2 * i:2 * i + 2, sl],
                                start=(i == 0), stop=(i == 1),
                                perf_mode=DR,
                            )
                        nc.scalar.activation(
                            out=h[:, ht, :], in_=ph, func=AF.Gelu,
                            bias=b1[:, ht:ht + 1], scale=1.0 / WSCALE,
                        )
                    out.append(f)
                return out

            def ffn2_pieces(p, ch):
                """4 thunks: FFN2 DR chain + bias + residual -> r2 + sq2."""
                io = ios[p]
                if "r2" not in io:
                    io["r2"] = pools[f"cd_{p}"].tile([P, CT, HW], BF16,
                                                     tag="r", name=f"r2_{p}")
                    io["sq2"] = pools[f"cd_{p}"].tile([P, CT, HW], FP8,
                                                      tag="sq", name=f"sq2_{p}")
                r2, sq2 = io["r2"], io["sq2"]
                h, w2 = io["h"], io["w2"]
                b2 = pp(p, "b2")
                sl = slice(ch * 512, (ch + 1) * 512)
                out = []
                for ct in range(CT):
                    def f(ct=ct):
                        pf = pools["psum_mm"].tile([P, 512], F32, tag="mm",
                                                   name="mm")
                        for i in range(HT // 2):
                            nc.tensor.matmul(
                                pf,
                                lhsT=w2[:, i, :, ct * P:(ct + 1) * P],
                                rhs=h[:, 2 * i:2 * i + 2, :],
                                start=(i == 0), stop=(i == HT // 2 - 1),
                                perf_mode=DR,
                            )
                        nc.vector.scalar_tensor_tensor(
                            out=r2[:, ct, sl], in0=pf,
                            scalar=b2[:, ct:ct + 1], in1=io["s"][:, ct, sl],
                            op0=ALU.add, op1=ALU.add,
                        )
                        nc.gpsimd.tensor_tensor(
                            sq2[:, ct, sl], r2[:, ct, sl], r2[:, ct, sl],
                            ALU.mult,
                        )
                    out.append(f)
                return out

            def ln2_finish(p, ch, pmu, pms):
                io = ios[p]
                w_ap, b_ap = pp(p, "n2w"), pp(p, "n2b")
                mu_bf, rs_bf = ln_rows(pmu, pms)

                def writer(ct, sl, t2):
                    o32 = pools["ostage"].tile([P, 512], F32, tag="o32",
                                               name="o32")
                    nc.vector.tensor_scalar(
                        out=o32, in0=t2,
                        scalar1=w_ap[:, ct:ct + 1], scalar2=b_ap[:, ct:ct + 1],
                        op0=ALU.mult, op1=ALU.add,
                    )
                    nc.sync.dma_start(out=outs[p][ct * P:(ct + 1) * P, sl],
                                      in_=o32)

                ln_norm(io["r2"], mu_bf, rs_bf, ch, writer)

            def load_wffn(p):
                # shared tags: the f-stream load waits for the s-stream's
                # last read, halving peak SBUF for FFN weights
                w1 = pools["wffn"].tile([P, 2, 2, HID], FP8, tag="w1",
                                        name=f"w1_{p}")
                nc.gpsimd.dma_start(
                    out=w1.rearrange("p a b o -> p (a b o)"), in_=w18[p])
                w2 = pools["wffn"].tile([P, 8, 2, C], FP8, tag="w2",
                                        name=f"w2_{p}")
                nc.scalar.dma_start(
                    out=w2.rearrange("p a b o -> p (a b o)"), in_=w28[p])
                ios[p]["w1"], ios[p]["w2"] = w1, w2

            def load_wo(p):
                nc.gpsimd.dma_start(
                    out=wo_sb[p],
                    in_=wot[p].rearrange("(ct p) o -> p ct o", p=P),
                )

            # ------------------------------------------------------------------
            # schedule
            # ------------------------------------------------------------------

            # prologue: x_s fp8 (ch-major so ch0 lands first), wq_s -> Q(s)
            load_x8("s")
            wq_s = load_w("wq", "s", wq8["s"])
            emit_proj_qk(x8_sb["s"], wq_s, qkv["q_s"])

            load_x8("f")
            wk_s = load_w("wk", "s", wk8["s"])
            emit_proj_qk(x8_sb["f"], wk_s, qkv["k_s"])
            wv_s = load_w("wv", "s", wv8["s"])
            emit_proj_v(x8_sb["f"], wv_s, qkv["v_s"])

            # residual inputs (bf16) trickle in behind the fp8 traffic
            load_xbf("s")
            load_xbf("f")

            # A(f) pieces as fillers inside early s-units
            def qf():
                emit_proj_qk(x8_sb["f"], load_w("wq", "f", wq8["f"]),
                             qkv["q_f"])

            def kf():
                emit_proj_qk(x8_sb["s"], load_w("wk", "f", wk8["f"]),
                             qkv["k_f"])

            def vf():
                emit_proj_v(x8_sb["s"], load_w("wv", "f", wv8["f"]),
                            qkv["v_f"])

            st("s", 0, [qf])
            st("s", 1, [kf])
            av("s", 0)
            st("s", 2, [vf, lambda: load_wo("s"), lambda: load_wo("f")])
            av("s", 1)
            st("s", 3)
            av("s", 2)

            # ---- f attention with s-stream C/D as fillers ----
            # ACT table discipline: LN1(s)'s sqrts form one burst; the 32
            # FFN1(s) gelus form one burst. Scattering them between exps
            # would cost 1283ns per exp<->sqrt/gelu table switch.
            io_s = ios["s"]

            def av_s3():
                av("s", 3)

            wo_s = wo_residual_pieces("s")
            st("f", 0, [av_s3] + wo_s)

            def ln1s_both():
                pmu, pms = ln_stats(io_s["r"], io_s["sq1"], 0)
                ln1_finish("s", 0, pmu, pms)
                pmu, pms = ln_stats(io_s["r"], io_s["sq1"], 1)
                ln1_finish("s", 1, pmu, pms)

            st("f", 1, [lambda: load_wffn("s"), ln1s_both])
            av("f", 0)

            def ffn1s_both():
                for f in ffn1_pieces("s", 0):
                    f()
                io_s["h_c0"] = io_s.pop("h")
                for f in ffn1_pieces("s", 1):
                    f()
                io_s["h_c1"] = io_s.pop("h")

            st("f", 2, [ffn1s_both])
            av("f", 1)

            io_s["h"] = io_s["h_c0"]
            f2c0 = ffn2_pieces("s", 0)
            io_s["h"] = io_s["h_c1"]
            f2c1 = ffn2_pieces("s", 1)
            st("f", 3, f2c0 + f2c1[:3])
            av("f", 2)
            for f in f2c1[3:]:
                f()
            av("f", 3)

            # ---- tail: f C/D + deferred LN2(s) (clusters sqrt/gelu tables) --
            for f in wo_residual_pieces("f"):
                f()
            load_wffn("f")
            io_f = ios["f"]

            # sqrt x4 cluster: ln2(s) + ln1(f)
            pmu_a, pms_a = ln_stats(io_s["r2"], io_s["sq2"], 0)
            ln2_finish("s", 0, pmu_a, pms_a)
            pmu_b, pms_b = ln_stats(io_s["r2"], io_s["sq2"], 1)
            ln2_finish("s", 1, pmu_b, pms_b)
            pmu_c, pms_c = ln_stats(io_f["r"], io_f["sq1"], 0)
            ln1_finish("f", 0, pmu_c, pms_c)
            pmu_d, pms_d = ln_stats(io_f["r"], io_f["sq1"], 1)
            ln1_finish("f", 1, pmu_d, pms_d)

            # FFN(f): both gelu chunks adjacent (single table window)
            for f in ffn1_pieces("f", 0):
                f()
            for f in ffn2_pieces("f", 0):
                f()
            io_f.pop("h")
            for f in ffn1_pieces("f", 1):
                f()
            for f in ffn2_pieces("f", 1):
                f()

            pmu_e, pms_e = ln_stats(io_f["r2"], io_f["sq2"], 0)
            ln2_finish("f", 0, pmu_e, pms_e)
            pmu_g, pms_g = ln_stats(io_f["r2"], io_f["sq2"], 1)
            ln2_finish("f", 1, pmu_g, pms_g)

    nc.compile()
    return nc


# --------------------------------------------------------------------------
# host side
# --------------------------------------------------------------------------

_BF = ml_dtypes.bfloat16
_F8 = ml_dtypes.float8_e4m3


def _kin_dr_layout(WT, scale=1.0):
    """W^T [C_in, C_out] -> [p, i, j, C_out] fp8 (contraction pair layout)."""
    nin = WT.shape[0]
    A = (WT * scale).reshape(nin // 256, 2, P, -1)   # (i, j, p, o)
    A = A.transpose(2, 0, 1, 3)                      # (p, i, j, o)
    return np.ascontiguousarray(A.reshape(P, -1)).astype(_F8)


def _prep_shared_inputs(inputs):
    sh = {}
    pcols = np.zeros((P, 2 * NPCOL), np.float32)
    for si, (p, ap) in enumerate((("s", "s_"), ("f", "f_"))):
        base = si * NPCOL
        wq, wk, wv, wo = (inputs[ap + n] for n in ("Wq", "Wk", "Wv", "Wo"))
        sh[f"{p}_wq8"] = _kin_dr_layout(np.ascontiguousarray(wq.T), WSCALE)
        sh[f"{p}_wk8"] = _kin_dr_layout(np.ascontiguousarray(wk.T), WSCALE)
        sh[f"{p}_wv8"] = _kin_dr_layout(np.ascontiguousarray(wv.T), WSCALE)
        sh[f"{p}_wot"] = np.ascontiguousarray(wo.T).astype(_F8)
        w1 = inputs[f"{p}ffn_W1"]
        w2 = inputs[f"{p}ffn_W2"]
        sh[f"{p}_w18"] = _kin_dr_layout(np.ascontiguousarray(w1.T), WSCALE)
        sh[f"{p}_w28"] = _kin_dr_layout(np.ascontiguousarray(w2.T))
        pcols[:, base + PCOL["bo"]:base + PCOL["bo"] + 4] = \
            inputs[ap + "bo"].reshape(CT, P).T
        for nm, src in (("n1w", f"{p}n1_w"), ("n1b", f"{p}n1_b"),
                        ("n2w", f"{p}n2_w"), ("n2b", f"{p}n2_b")):
            pcols[:, base + PCOL[nm]:base + PCOL[nm] + 4] = \
                inputs[src].reshape(CT, P).T
        pcols[:, base + PCOL["b2"]:base + PCOL["b2"] + 4] = \
            inputs[f"{p}ffn_b2"].reshape(CT, P).T
        pcols[:, base + PCOL["b1"]:base + PCOL["b1"] + 16] = \
            inputs[f"{p}ffn_b1"].reshape(HT, P).T
    sh["prm"] = pcols
    return sh


def make_in_maps(inputs):
    shared = _prep_shared_inputs(inputs)
    xs = np.ascontiguousarray(inputs["spatial_feat"].reshape(B, C, HW))
    xf = np.ascontiguousarray(inputs["freq_feat"].reshape(B, C, HW))
    in_maps = []
    for b in range(N_CORES):
        m = dict(shared)
        m["x_s8"] = xs[b].astype(_F8)
        m["x_f8"] = xf[b].astype(_F8)
        m["x_sbf"] = xs[b].astype(_BF)
        m["x_fbf"] = xf[b].astype(_BF)
        in_maps.append(m)
    return in_maps


_CACHED = {}


def _get_program():
    if "nc" not in _CACHED:
        _CACHED["nc"] = build_program()
    return _CACHED["nc"]


def run_on_hw(inputs, trace=False, trace_kwargs=None):
    from concourse.bass_utils import run_bass_kernel_spmd

    nc = _get_program()
    in_maps = make_in_maps(inputs)
    res = run_bass_kernel_spmd(
        nc, in_maps, list(range(N_CORES)), trace=trace,
        **(dict(trace_kwargs=trace_kwargs) if trace_kwargs else {}),
    )
    s = np.stack([res.results[b]["out_s"] for b in range(B)])
    f = np.stack([res.results[b]["out_f"] for b in range(B)])
    s = s.reshape(B, C, H_IMG, W_IMG).astype(np.float32)
    f = f.reshape(B, C, H_IMG, W_IMG).astype(np.float32)
    return (s, f), res


def kernel(**inputs):
    out, _ = run_on_hw(inputs, trace=False)
    return out
